# revision 4
# baseline (speedup 1.0000x reference)
"""Trainium2 Bass kernel for nn_DCTLayer: per-8x8-block 2D DCT-like transform.

Math: reference computes, per 8x8 block X of the 256x256 image,
    out_block[y, v] = sum_x A[v, x] * X[x, y],   where A = D @ D
(D = 8x8 DCT basis). out_block = (A @ X)^T.

v2 strategy (fp16 end-to-end; correctness gate is 2e-2, fp16 gives ~5e-4):
  - Host casts x to fp16 -> HBM traffic halves (DMA was 89% busy at fp32).
  - Per core, pure data parallel over batch: 128 images of 256x256.
  - Load GI=8 images per DMA: xt[p=(G,x), (q=img*2+r, c)] fp16,
    rows = 128*q + 8*G + x, cols c = 8*J + y.
  - ONE fp16 matmul per image, stationary = 128x128 block-diagonal BD
    (16 copies of A^T, columns permuted so out partition = (G3G2|v|G1G0)):
    ps[(G3G2,v,G1G0), (r,J,y)] = (A @ X_block)[v, y]  (PSUM fp32).
  - SCALAR engine evicts PSUM -> SBUF with fp32->fp16 cast (s0), freeing
    the DVE (was 89% busy) from the PSUM-read bubble + fp32 streams.
  - DVE T1 (per r-half): 32x32 stream transpose,
      in  s0 [p, y@1, J@8] -> out s2 [p, y@8, v@1, G1G0@64]
      partition (G3G2|v,G1G0) -> (G3G2|J5); s2 phys (G1G0, y, v), v@1.
  - DVE T2 (whole image, int32-packed: v-pairs ride as one int32 ->
    half the streamed elements):
      in  s2.i32 [p, r@128, vh@1, g@4] (g = (G1G0,y) merged, uniform)
      out zt.i32 [p, r@128, vh@1, J@4]
      partition (G3G2|J5) -> (G,y); zt phys (r, J, v) = contiguous rows.
  - Store 8 images per DMA (fp16), host casts back to fp32.
"""

import sys

sys.path.insert(0, "/opt/trn_rl_repo")

from contextlib import ExitStack

import numpy as np

import concourse.bass as bass  # noqa: F401
import concourse.tile as tile
from concourse import bacc, mybir
from concourse.bass_utils import run_bass_kernel_spmd

P = 8
H = W = 256
B, C = 16, 64
NCORES = 8
BPC = B // NCORES  # batches per core
IMGS = BPC * C  # images (b,c planes) per core
ROWS = IMGS * H  # dram rows per core
GI = 8  # images per DMA group
NGRP = IMGS // GI

TRACE = False
LAST_RESULTS = None

_nc_cache = None


def _ensure_ntff_hook():
    """The agent image's antenv lacks axon_hooks; synthesize it so
    run_bass_kernel_spmd(trace=True) can capture NTFF profiles."""
    import types

    if "antenv.axon_hooks" in sys.modules:
        return
    try:
        sys.path.insert(0, "/root/.axon_site/trn_agent_boot")
        from trn_boot import _ntff_profile_via_ctypes

        hook = _ntff_profile_via_ctypes("/opt/axon/libaxon_pjrt.so")
    except Exception:
        hook = None
    mod = types.ModuleType("antenv.axon_hooks")
    mod._hook = hook
    mod.get_axon_ntff_profile_hook = lambda: mod._hook
    mod.set_axon_ntff_profile_hook = lambda h: setattr(mod, "_hook", h)
    sys.modules["antenv.axon_hooks"] = mod


def _stream_transpose(nc, out_ap, in_ap):
    """nc.vector.transpose but with opt=False AP lowering: the AP dim
    order IS the stream order for InstStreamTranspose, so the optimizer
    must not merge/reorder dims."""
    eng = nc.vector
    return eng.add_instruction(
        mybir.InstStreamTranspose(
            name=eng.bass.get_next_instruction_name(),
            ins=[eng.lower_ap(in_ap, opt=False)],
            outs=[eng.lower_ap(out_ap, opt=False)],
        )
    )


def _dct_kernel(tc, o, x, bd):
    nc = tc.nc
    f16 = mybir.dt.float16
    i32 = mybir.dt.int32
    with ExitStack() as ctx:
        xpool = ctx.enter_context(tc.tile_pool(name="xin", bufs=3))
        s0pool = ctx.enter_context(tc.tile_pool(name="s0", bufs=6))
        s2pool = ctx.enter_context(tc.tile_pool(name="s2", bufs=6))
        zpool = ctx.enter_context(tc.tile_pool(name="zout", bufs=3))
        cpool = ctx.enter_context(tc.tile_pool(name="const", bufs=1))
        ppool = ctx.enter_context(tc.tile_pool(name="ps", bufs=8, space="PSUM"))

        bdt = cpool.tile([128, 128], f16)
        nc.sync.dma_start(bdt[:], bd[:])

        for g in range(NGRP):
            # ---- load 8 images (16 x 128-row chunks) ----
            xt = xpool.tile([128, GI * 2 * W], f16)
            src = x[g * GI * H : (g + 1) * GI * H, :].rearrange(
                "(q p) c -> p q c", p=128
            )
            dst = xt[:].rearrange("p (q c) -> p q c", c=W)
            nc.sync.dma_start(dst, src)

            zt = zpool.tile([128, GI * 2 * W], f16)
            for i in range(GI):
                xi = xt[:, i * 512 : (i + 1) * 512]
                # ---- one fp16 matmul: BD stationary, data moving ----
                ps = ppool.tile([128, 512], mybir.dt.float32)
                nc.tensor.matmul(ps[:], bdt[:], xi, start=True, stop=True)

                # ---- DVE T1, one instr (fp32, PSUM->SBUF; 4B dtype
                # tolerates strided streams at full rate): ----
                # s1 phys: (r@256, G1G0@64, y@8, v@1) fp32
                s1 = s0pool.tile([128, 512], mybir.dt.float32)
                tin = ps[:].rearrange("p (r J y) -> p y r J", r=2, J=32, y=8)
                tout = s1[:].rearrange(
                    "p (r G y v) -> p y r v G", r=2, G=4, y=8, v=8
                )
                _stream_transpose(nc, tout, tin)

                # ---- scalar engine: contiguous cast fp32 -> fp16 ----
                s2 = s2pool.tile([128, 512], f16)
                nc.scalar.copy(s2[:], s1[:])

                # ---- DVE T2 (int32-packed): J-part <-> (G1G0,y)-stream ----
                # s2.i32 layout: (r@128, G1G0@32, y@4, vh@1); g=(G1G0,y)@4
                tin2 = (
                    s2[:]
                    .bitcast(i32)
                    .rearrange("p (r G y vh) -> p r vh (G y)", r=2, G=4, y=8, vh=4)
                )
                tout2 = (
                    zt[:, i * 512 : (i + 1) * 512]
                    .bitcast(i32)
                    .rearrange("p (r J vh) -> p r vh J", r=2, J=32, vh=4)
                )
                _stream_transpose(nc, tout2, tin2)

            # ---- store 8 images ----
            dsto = o[g * GI * H : (g + 1) * GI * H, :].rearrange(
                "(q p) c -> p q c", p=128
            )
            srco = zt[:].rearrange("p (q c) -> p q c", c=W)
            nc.scalar.dma_start(dsto, srco)


def _build_nc():
    nc = bacc.Bacc(
        "TRN2", target_bir_lowering=False, debug=False, num_devices=NCORES
    )
    x_ap = nc.dram_tensor(
        "x", [ROWS, W], mybir.dt.float16, kind="ExternalInput"
    ).ap()
    bd_ap = nc.dram_tensor(
        "bd", [128, 128], mybir.dt.float16, kind="ExternalInput"
    ).ap()
    o_ap = nc.dram_tensor(
        "o", [ROWS, W], mybir.dt.float16, kind="ExternalOutput"
    ).ap()
    with tile.TileContext(nc) as tc:
        _dct_kernel(tc, o_ap, x_ap, bd_ap)
    nc.compile()
    return nc


def _make_bd(dct_basis: np.ndarray) -> np.ndarray:
    """Block-diagonal A^T with columns permuted so the matmul's output
    partition index is (G3G2 | v2v1v0 | G1G0) instead of (G4 | v3)."""
    a = dct_basis.astype(np.float64) @ dct_basis.astype(np.float64)
    at = a.T  # at[x, v] = A[v, x]
    bd = np.zeros((128, 128), dtype=np.float64)
    for g in range(16):
        for v in range(P):
            m = (g >> 2) * 32 + v * 4 + (g & 3)
            bd[g * P : (g + 1) * P, m] = at[:, v]
    return bd.astype(np.float16)


def kernel(x: np.ndarray, dct_basis: np.ndarray) -> np.ndarray:
    global _nc_cache, LAST_RESULTS
    x = np.asarray(x)
    dct_basis = np.asarray(dct_basis, dtype=np.float32)
    assert x.shape == (B, C, H, W)

    if _nc_cache is None:
        _nc_cache = _build_nc()
    nc = _nc_cache

    bd = _make_bd(dct_basis)
    xh = np.ascontiguousarray(x).astype(np.float16)
    in_maps = []
    for i in range(NCORES):
        xs = xh[i * BPC : (i + 1) * BPC].reshape(ROWS, W)
        in_maps.append({"x": xs, "bd": bd})

    if TRACE:
        _ensure_ntff_hook()
    try:
        res = run_bass_kernel_spmd(
            nc, in_maps, core_ids=list(range(NCORES)), trace=TRACE
        )
    except ModuleNotFoundError:
        res = run_bass_kernel_spmd(
            nc, in_maps, core_ids=list(range(NCORES)), trace=False
        )
    LAST_RESULTS = res

    out = np.empty((B, C, H, W), dtype=np.float32)
    for i in range(NCORES):
        out[i * BPC : (i + 1) * BPC] = (
            res.results[i]["o"].astype(np.float32).reshape(BPC, C, H, W)
        )
    return out


# revision 7
# speedup vs baseline: 1.0140x; 1.0140x over previous
"""Trainium2 Bass kernel for nn_DCTLayer: per-8x8-block 2D DCT-like transform.

Math: reference computes, per 8x8 block X of the 256x256 image,
    out_block[y, v] = sum_x A[v, x] * X[x, y],   where A = D @ D
(D = 8x8 DCT basis). out_block = (A @ X)^T.

v2 strategy (fp16 end-to-end; correctness gate is 2e-2, fp16 gives ~5e-4):
  - Host casts x to fp16 -> HBM traffic halves (DMA was 89% busy at fp32).
  - Per core, pure data parallel over batch: 128 images of 256x256.
  - Load GI=8 images per DMA: xt[p=(G,x), (q=img*2+r, c)] fp16,
    rows = 128*q + 8*G + x, cols c = 8*J + y.
  - ONE fp16 matmul per image, stationary = 128x128 block-diagonal BD
    (16 copies of A^T, columns permuted so out partition = (G3G2|v|G1G0)):
    ps[(G3G2,v,G1G0), (r,J,y)] = (A @ X_block)[v, y]  (PSUM fp32).
  - SCALAR engine evicts PSUM -> SBUF with fp32->fp16 cast (s0), freeing
    the DVE (was 89% busy) from the PSUM-read bubble + fp32 streams.
  - DVE T1 (per r-half): 32x32 stream transpose,
      in  s0 [p, y@1, J@8] -> out s2 [p, y@8, v@1, G1G0@64]
      partition (G3G2|v,G1G0) -> (G3G2|J5); s2 phys (G1G0, y, v), v@1.
  - DVE T2 (whole image, int32-packed: v-pairs ride as one int32 ->
    half the streamed elements):
      in  s2.i32 [p, r@128, vh@1, g@4] (g = (G1G0,y) merged, uniform)
      out zt.i32 [p, r@128, vh@1, J@4]
      partition (G3G2|J5) -> (G,y); zt phys (r, J, v) = contiguous rows.
  - Store 8 images per DMA (fp16), host casts back to fp32.
"""

import sys

sys.path.insert(0, "/opt/trn_rl_repo")

from contextlib import ExitStack

import numpy as np

import concourse.bass as bass  # noqa: F401
import concourse.tile as tile
from concourse import bacc, mybir
from concourse.bass_utils import run_bass_kernel_spmd

P = 8
H = W = 256
B, C = 16, 64
NCORES = 8
BPC = B // NCORES  # batches per core
IMGS = BPC * C  # images (b,c planes) per core
ROWS = IMGS * H  # dram rows per core
GI = 8  # images per DMA group
NGRP = IMGS // GI

TRACE = False
LAST_RESULTS = None

_nc_cache = None


def _ensure_ntff_hook():
    """The agent image's antenv lacks axon_hooks; synthesize it so
    run_bass_kernel_spmd(trace=True) can capture NTFF profiles."""
    import types

    if "antenv.axon_hooks" in sys.modules:
        return
    try:
        sys.path.insert(0, "/root/.axon_site/trn_agent_boot")
        from trn_boot import _ntff_profile_via_ctypes

        hook = _ntff_profile_via_ctypes("/opt/axon/libaxon_pjrt.so")
    except Exception:
        hook = None
    mod = types.ModuleType("antenv.axon_hooks")
    mod._hook = hook
    mod.get_axon_ntff_profile_hook = lambda: mod._hook
    mod.set_axon_ntff_profile_hook = lambda h: setattr(mod, "_hook", h)
    sys.modules["antenv.axon_hooks"] = mod


def _stream_transpose(nc, out_ap, in_ap):
    """nc.vector.transpose but with opt=False AP lowering: the AP dim
    order IS the stream order for InstStreamTranspose, so the optimizer
    must not merge/reorder dims."""
    eng = nc.vector
    return eng.add_instruction(
        mybir.InstStreamTranspose(
            name=eng.bass.get_next_instruction_name(),
            ins=[eng.lower_ap(in_ap, opt=False)],
            outs=[eng.lower_ap(out_ap, opt=False)],
        )
    )


def _dct_kernel(tc, o, x, bd):
    nc = tc.nc
    f16 = mybir.dt.float16
    i32 = mybir.dt.int32
    with ExitStack() as ctx:
        xpool = ctx.enter_context(tc.tile_pool(name="xin", bufs=3))
        s0pool = ctx.enter_context(tc.tile_pool(name="s0", bufs=6))
        s2pool = ctx.enter_context(tc.tile_pool(name="s2", bufs=6))
        zpool = ctx.enter_context(tc.tile_pool(name="zout", bufs=3))
        cpool = ctx.enter_context(tc.tile_pool(name="const", bufs=1))
        ppool = ctx.enter_context(tc.tile_pool(name="ps", bufs=8, space="PSUM"))

        bdt = cpool.tile([128, 128], f16)
        nc.sync.dma_start(bdt[:], bd[:])

        for g in range(NGRP):
            # ---- load 8 images (16 x 128-row chunks) ----
            xt = xpool.tile([128, GI * 2 * W], f16)
            src = x[g * GI * H : (g + 1) * GI * H, :].rearrange(
                "(q p) c -> p q c", p=128
            )
            dst = xt[:].rearrange("p (q c) -> p q c", c=W)
            nc.sync.dma_start(dst, src)

            zt = zpool.tile([128, GI * 2 * W], f16)
            for i2 in range(GI // 2):
                # ---- process image PAIRS to halve per-instr overheads ----
                s2 = s2pool.tile([128, 1024], f16)
                for k in range(2):
                    i = i2 * 2 + k
                    xi = xt[:, i * 512 : (i + 1) * 512]
                    # ---- one fp16 matmul: BD stationary, data moving ----
                    ps = ppool.tile([128, 512], mybir.dt.float32)
                    nc.tensor.matmul(ps[:], bdt[:], xi, start=True, stop=True)

                    # ---- DVE T1, one instr (fp32, PSUM->SBUF; 4B dtype
                    # tolerates strided streams at full rate): ----
                    # s1 phys: (r@256, G1G0@64, y@8, v@1) fp32
                    s1 = s0pool.tile([128, 512], mybir.dt.float32)
                    tin = ps[:].rearrange(
                        "p (r J y) -> p y r J", r=2, J=32, y=8
                    )
                    tout = s1[:].rearrange(
                        "p (r G y v) -> p y r v G", r=2, G=4, y=8, v=8
                    )
                    _stream_transpose(nc, tout, tin)

                    # ---- scalar engine: contiguous cast fp32 -> fp16 ----
                    nc.scalar.copy(s2[:, k * 512 : (k + 1) * 512], s1[:])

                # ---- DVE T2 for the pair (int32-packed):
                # J-part <-> (G1G0,y)-stream; per img, s2.i32 layout
                # (r@128, G1G0@32, y@4, vh@1); g=(G1G0,y)@4 ----
                tin2 = (
                    s2[:]
                    .bitcast(i32)
                    .rearrange(
                        "p (k r G y vh) -> p (k r) vh (G y)",
                        k=2, r=2, G=4, y=8, vh=4,
                    )
                )
                tout2 = (
                    zt[:, i2 * 1024 : (i2 + 1) * 1024]
                    .bitcast(i32)
                    .rearrange(
                        "p (k r J vh) -> p (k r) vh J", k=2, r=2, J=32, vh=4
                    )
                )
                _stream_transpose(nc, tout2, tin2)

            # ---- store 8 images ----
            dsto = o[g * GI * H : (g + 1) * GI * H, :].rearrange(
                "(q p) c -> p q c", p=128
            )
            srco = zt[:].rearrange("p (q c) -> p q c", c=W)
            nc.scalar.dma_start(dsto, srco)


def _build_nc():
    nc = bacc.Bacc(
        "TRN2", target_bir_lowering=False, debug=False, num_devices=NCORES
    )
    x_ap = nc.dram_tensor(
        "x", [ROWS, W], mybir.dt.float16, kind="ExternalInput"
    ).ap()
    bd_ap = nc.dram_tensor(
        "bd", [128, 128], mybir.dt.float16, kind="ExternalInput"
    ).ap()
    o_ap = nc.dram_tensor(
        "o", [ROWS, W], mybir.dt.float16, kind="ExternalOutput"
    ).ap()
    with tile.TileContext(nc) as tc:
        _dct_kernel(tc, o_ap, x_ap, bd_ap)
    nc.compile()
    return nc


def _make_bd(dct_basis: np.ndarray) -> np.ndarray:
    """Block-diagonal A^T with columns permuted so the matmul's output
    partition index is (G3G2 | v2v1v0 | G1G0) instead of (G4 | v3)."""
    a = dct_basis.astype(np.float64) @ dct_basis.astype(np.float64)
    at = a.T  # at[x, v] = A[v, x]
    bd = np.zeros((128, 128), dtype=np.float64)
    for g in range(16):
        for v in range(P):
            m = (g >> 2) * 32 + v * 4 + (g & 3)
            bd[g * P : (g + 1) * P, m] = at[:, v]
    return bd.astype(np.float16)


def kernel(x: np.ndarray, dct_basis: np.ndarray) -> np.ndarray:
    global _nc_cache, LAST_RESULTS
    x = np.asarray(x)
    dct_basis = np.asarray(dct_basis, dtype=np.float32)
    assert x.shape == (B, C, H, W)

    if _nc_cache is None:
        _nc_cache = _build_nc()
    nc = _nc_cache

    bd = _make_bd(dct_basis)
    xh = np.ascontiguousarray(x).astype(np.float16)
    in_maps = []
    for i in range(NCORES):
        xs = xh[i * BPC : (i + 1) * BPC].reshape(ROWS, W)
        in_maps.append({"x": xs, "bd": bd})

    if TRACE:
        _ensure_ntff_hook()
    try:
        res = run_bass_kernel_spmd(
            nc, in_maps, core_ids=list(range(NCORES)), trace=TRACE
        )
    except ModuleNotFoundError:
        res = run_bass_kernel_spmd(
            nc, in_maps, core_ids=list(range(NCORES)), trace=False
        )
    LAST_RESULTS = res

    out = np.empty((B, C, H, W), dtype=np.float32)
    for i in range(NCORES):
        out[i * BPC : (i + 1) * BPC] = (
            res.results[i]["o"].astype(np.float32).reshape(BPC, C, H, W)
        )
    return out
